# revision 1
# baseline (speedup 1.0000x reference)
"""Multi-modal causal cross-attention + MLP on 8 Trainium2 NeuronCores.

Problem (hardcoded): B=4, S=2048, C=1024, H=16, HS=64, M=3.
    q = einsum('bsc,hcd->bhsd', query_x, Wq)
    per modality m: kv = einsum('bsc,hcd->bhsd', kv_x[m], Wkv[m]); k, v = split
    out += causal-softmax(q k^T / sqrt(hs)) @ v     (summed over m)
    y = tanh(concat-heads(out) @ W1 + b1) @ W2 + b2

Sharding: data-parallel over batch (4) x tensor-parallel over heads (2 groups
of 8), one (batch, head-group) per core.  The head-group partial of the first
MLP matmul (y1) is returned per core; the host adds the two partials per batch
(8.4 MFLOP of glue) and a second tiny launch does tanh + W2 on per-core
q-halves.  (In-kernel cross-core collectives do not load under this runtime.)

Device layout notes (launch A, per core):
  - x is transposed to xT [c, s] via HWDGE DMA-transpose (bf16).
  - qT/kT [d, s] head-pair-packed on 128 partitions; scores computed as
    scoresT [k, q] = kT.T @ qT with two heads row-packed on the PE array.
  - v is projected directly into [s, h*d] layout (no transpose), stored with a
    per-head ones column: the attn@v matmul (M=65) then yields both the
    attention output and the softmax denominator in one stream.
  - exp on ScalarE (psum -> sbuf bf16), causal masking only on diagonal
    128x128 blocks via a triangular 0/1 mask multiply on VectorE.
  - normalization (x 1/denom broadcast along partitions via a K=1 matmul)
    is fused into the mandatory psum->sbuf copy (scalar_tensor_tensor).
  - W1 partial: y1_partial[j, q] accumulated over (head, modality) in psum.
"""

import numpy as np
import ml_dtypes

import concourse.bass as bass
import concourse.tile as tile
from concourse import bacc, mybir
from concourse.bass_utils import run_bass_kernel_spmd

BF = ml_dtypes.bfloat16
F32 = mybir.dt.float32
F32R = mybir.dt.float32r
BF16 = mybir.dt.bfloat16

B, S, C, H, HS, M3 = 4, 2048, 1024, 16, 64, 3
N_CORES = 8
EXP = mybir.ActivationFunctionType.Exp
TANH = mybir.ActivationFunctionType.Tanh
MULT = mybir.AluOpType.mult

_CACHE = {}


def _build_launch_a():
    nc = bacc.Bacc("TRN2", target_bir_lowering=False, debug=False, num_devices=N_CORES)
    xq = nc.dram_tensor("xq", [S, C], BF16, kind="ExternalInput").ap()
    xkv = nc.dram_tensor("xkv", [M3, S, C], BF16, kind="ExternalInput").ap()
    wq = nc.dram_tensor("wq", [8, 128, 512], BF16, kind="ExternalInput").ap()
    wk = nc.dram_tensor("wk", [M3, 8, 128, 512], BF16, kind="ExternalInput").ap()
    wv = nc.dram_tensor("wv", [M3, 8, 128, 512], BF16, kind="ExternalInput").ap()
    w1 = nc.dram_tensor("w1", [64, 8, 4, 128], BF16, kind="ExternalInput").ap()
    tri = nc.dram_tensor("tri", [128, 128], BF16, kind="ExternalInput").ap()
    y1p = nc.dram_tensor("y1p", [4, 128, S], F32, kind="ExternalOutput").ap()

    with tile.TileContext(nc) as tc:
        import contextlib
        with contextlib.ExitStack() as stk:
            singles = stk.enter_context(tc.tile_pool(name="singles", bufs=1))
            qT_sb = singles.tile([128, 4, S], BF16, tag="qT")
            kT_sb = singles.tile([128, M3, 4, S], BF16, tag="kT")
            vE_sb = singles.tile([128, M3, 16, 8, 65], BF16, tag="vE")
            w1_sb = singles.tile([128, 8, 4, 128], BF16, tag="w1")
            tri_sb = singles.tile([128, 128], BF16, tag="tri")
            av_sb = singles.tile([128, M3, 8, 512], BF16, tag="av")

            nc.vector.memset(vE_sb[:, :, :, :, 64:65], 1.0)
            nc.sync.dma_start(out=tri_sb[:], in_=tri[:])
            nc.sync.dma_start(out=w1_sb[0:64, :, :, :], in_=w1[:])

            # ---------------- Phase 1: projections ----------------
            with tc.tile_pool(name="wts", bufs=2) as wts, \
                 tc.tile_pool(name="xtp", bufs=4) as xtp, \
                 tc.tile_pool(name="pp", bufs=8, space="PSUM") as pp:

                # query -> qT (head-pair packed [d2, s])
                wq_sb = wts.tile([128, 8, 512], BF16, tag="wqk")
                nc.sync.dma_start(out=wq_sb[:], in_=wq.rearrange("c p j -> p c j"))
                for sb in range(4):
                    ps = [pp.tile([128, 512], F32, tag="acc", name=f"psq{i}") for i in range(4)]
                    for c in range(8):
                        xt = xtp.tile([128, 512], BF16, tag="xt")
                        nc.sync.dma_start(
                            out=xt[:],
                            in_=xq[sb * 512:(sb + 1) * 512, c * 128:(c + 1) * 128],
                            transpose=True,
                        )
                        for p in range(4):
                            nc.tensor.matmul(
                                ps[p][:], wq_sb[:, c, p * 128:(p + 1) * 128], xt[:],
                                start=(c == 0), stop=(c == 7),
                            )
                    for p in range(4):
                        nc.vector.tensor_copy(qT_sb[:, p, sb * 512:(sb + 1) * 512], ps[p][:])

                # kv per modality -> kT (pair packed) and v_ext [s, h*65]
                for m in range(M3):
                    wk_sb = wts.tile([128, 8, 512], BF16, tag="wqk")
                    wv_sb = wts.tile([128, 8, 512], BF16, tag="wv")
                    nc.sync.dma_start(out=wk_sb[:], in_=wk[m].rearrange("c p j -> p c j"))
                    nc.sync.dma_start(out=wv_sb[:], in_=wv[m].rearrange("c p j -> p c j"))
                    for sb in range(4):
                        psk = [pp.tile([128, 512], F32, tag="acc", name=f"psk{i}") for i in range(4)]
                        psv = [pp.tile([128, 8, 64], F32, tag="acc", name=f"psv{i}") for i in range(4)]
                        for c in range(8):
                            xt = xtp.tile([128, 512], BF16, tag="xt")
                            nc.sync.dma_start(
                                out=xt[:],
                                in_=xkv[m, sb * 512:(sb + 1) * 512, c * 128:(c + 1) * 128],
                                transpose=True,
                            )
                            for p in range(4):
                                nc.tensor.matmul(
                                    psk[p][:], wk_sb[:, c, p * 128:(p + 1) * 128], xt[:],
                                    start=(c == 0), stop=(c == 7),
                                )
                            for sc in range(4):
                                nc.tensor.matmul(
                                    psv[sc][:], xt[:, sc * 128:(sc + 1) * 128], wv_sb[:, c, :],
                                    start=(c == 0), stop=(c == 7),
                                )
                        for p in range(4):
                            nc.vector.tensor_copy(kT_sb[:, m, p, sb * 512:(sb + 1) * 512], psk[p][:])
                        for sc in range(4):
                            nc.vector.tensor_copy(
                                vE_sb[:, m, sb * 4 + sc, :, 0:64], psv[sc][:]
                            )

            # ---------------- Phase 2: attention + W1 partial ----------------
            with tc.tile_pool(name="probs", bufs=6) as probs_p, \
                 tc.tile_pool(name="rcp", bufs=4) as rcp_p, \
                 tc.tile_pool(name="rbp", bufs=4) as rb_p, \
                 tc.tile_pool(name="dsc", bufs=4, space="DRAM") as dsc_p, \
                 tc.tile_pool(name="y1s", bufs=3) as y1s_p, \
                 tc.tile_pool(name="ps_s", bufs=3, space="PSUM") as ps_s, \
                 tc.tile_pool(name="ps_av", bufs=3, space="PSUM") as ps_av, \
                 tc.tile_pool(name="ps_y1", bufs=2, space="PSUM") as ps_y1:

                for qt in range(4):
                    for m in range(M3):
                        for h in range(8):
                            hp, ho = h // 2, 64 * (h % 2)
                            pav = ps_av.tile([128, 512], F32, tag="pav")
                            nkt = 4 * (qt + 1)
                            for kt in range(nkt):
                                qs = kt - 4 * qt
                                c0 = max(qs, 0) * 128
                                psc = ps_s.tile([128, 512], F32, tag="psc")
                                nc.tensor.matmul(
                                    psc[:, c0:512],
                                    kT_sb[ho:ho + 64, m, hp, kt * 128:(kt + 1) * 128],
                                    qT_sb[ho:ho + 64, hp, qt * 512 + c0:(qt + 1) * 512],
                                    start=True, stop=True,
                                )
                                pr = probs_p.tile([128, 512], BF16, tag="pr")
                                nc.scalar.activation(pr[:, c0:512], psc[:, c0:512], EXP)
                                if qs >= 0:
                                    nc.vector.tensor_tensor(
                                        pr[:, c0:c0 + 128], pr[:, c0:c0 + 128], tri_sb[:], MULT
                                    )
                                nc.tensor.matmul(
                                    pav[0:65, c0:512],
                                    vE_sb[:, m, kt, h, :],
                                    pr[:, c0:512],
                                    start=(kt == 0), stop=(kt == nkt - 1),
                                    skip_group_check=True,
                                )
                            rc = rcp_p.tile([128, 512], F32, tag="rc")
                            nc.vector.reciprocal(rc[64:65, :], pav[64:65, :])
                            dscr = dsc_p.tile([1, 512], F32, tag="dscr")
                            nc.sync.dma_start(out=dscr[:], in_=rc[64:65, :])
                            rb = rb_p.tile([128, 512], F32, tag="rb")
                            nc.sync.dma_start(
                                out=rb[0:64, :],
                                in_=bass.AP(tensor=dscr.tensor, offset=dscr.offset,
                                            ap=[[0, 64], [1, 512]]),
                            )
                            nc.vector.tensor_tensor(
                                av_sb[0:64, m, h, :], pav[0:64, :], rb[0:64, :], MULT
                            )
                    for jc in range(4):
                        py1 = ps_y1.tile([128, 512], F32, tag="py1")
                        for m in range(M3):
                            for h in range(8):
                                nc.tensor.matmul(
                                    py1[:], w1_sb[0:64, h, jc, :], av_sb[0:64, m, h, :],
                                    start=(m == 0 and h == 0), stop=(m == M3 - 1 and h == 7),
                                )
                        y1t = y1s_p.tile([128, 512], F32, tag="y1t")
                        nc.vector.tensor_copy(y1t[:], py1[:])
                        nc.sync.dma_start(out=y1p[jc, :, qt * 512:(qt + 1) * 512], in_=y1t[:])
    nc.compile()
    return nc


def _build_launch_b():
    nc = bacc.Bacc("TRN2", target_bir_lowering=False, debug=False, num_devices=N_CORES)
    y1h = nc.dram_tensor("y1h", [128, 4, 1024], F32, kind="ExternalInput").ap()
    b1s = nc.dram_tensor("b1s", [128, 4], F32, kind="ExternalInput").ap()
    w2 = nc.dram_tensor("w2", [128, 4, 1024], BF16, kind="ExternalInput").ap()
    b2 = nc.dram_tensor("b2", [1, 1024], BF16, kind="ExternalInput").ap()
    ob = nc.dram_tensor("ob", [1024, 1024], F32, kind="ExternalOutput").ap()

    with tile.TileContext(nc) as tc:
        with tc.tile_pool(name="sg", bufs=1) as sg, \
             tc.tile_pool(name="ot", bufs=4) as ot_p, \
             tc.tile_pool(name="po", bufs=4, space="PSUM") as po_p:
            y1f = sg.tile([128, 4, 1024], F32, tag="y1f")
            y1t = sg.tile([128, 4, 1024], BF16, tag="y1tt")
            w2_sb = sg.tile([128, 4, 1024], BF16, tag="w2")
            b1_sb = sg.tile([128, 4], F32, tag="b1")
            b2_sb = sg.tile([128, 1024], BF16, tag="b2")
            ones_b = sg.tile([128, 128], BF16, tag="onesb")

            nc.sync.dma_start(out=y1f[:], in_=y1h[:])
            nc.sync.dma_start(out=w2_sb[:], in_=w2[:])
            nc.sync.dma_start(out=b1_sb[:], in_=b1s[:])
            nc.sync.dma_start(out=b2_sb[0:1, :], in_=b2[:])
            nc.vector.memset(ones_b[:], 1.0)

            for jc in range(4):
                nc.scalar.activation(
                    y1t[:, jc, :], y1f[:, jc, :], TANH, bias=b1_sb[:, jc:jc + 1], scale=1.0
                )
            for qc in range(8):
                for ch in range(2):
                    po = po_p.tile([128, 512], F32, tag="po")
                    for jc in range(4):
                        nc.tensor.matmul(
                            po[:], y1t[:, jc, qc * 128:(qc + 1) * 128],
                            w2_sb[:, jc, ch * 512:(ch + 1) * 512],
                            start=(jc == 0), stop=False, skip_group_check=True,
                        )
                    nc.tensor.matmul(
                        po[:], ones_b[0:1, 0:128], b2_sb[0:1, ch * 512:(ch + 1) * 512],
                        start=False, stop=True, skip_group_check=True,
                    )
                    o_t = ot_p.tile([128, 512], F32, tag="ot")
                    nc.vector.tensor_copy(o_t[:], po[:])
                    nc.sync.dma_start(
                        out=ob[qc * 128:(qc + 1) * 128, ch * 512:(ch + 1) * 512], in_=o_t[:]
                    )
    nc.compile()
    return nc


def _pack_wqk(w, scale=None):
    """[8, C, HS] per-head -> pair-packed [8 c-chunk, 128 c-in, 512 (pair*128)]."""
    if scale is not None:
        w = w * scale
    # [8h, C, 64] -> [C, 4 pair, 2, 64] -> [C, 4, 128]
    a = w.reshape(4, 2, C, HS).transpose(2, 0, 1, 3).reshape(C, 4, 128)
    return np.ascontiguousarray(a.reshape(8, 128, 512)).astype(BF)


def _pack_wv(w):
    """[8, C, HS] -> [8 c-chunk, 128 c-in, 512 (h*64)]."""
    a = w.transpose(1, 0, 2).reshape(C, 512)
    return np.ascontiguousarray(a.reshape(8, 128, 512)).astype(BF)


def kernel(query_x, kv_x, Wq, Wkv, W1, b1, W2, b2):
    if "a" not in _CACHE:
        _CACHE["a"] = _build_launch_a()
        _CACHE["b"] = _build_launch_b()
    nc_a, nc_b = _CACHE["a"], _CACHE["b"]

    query_x = np.asarray(query_x, dtype=np.float32)
    kv_x = np.asarray(kv_x, dtype=np.float32)
    Wq = np.asarray(Wq, dtype=np.float32)
    Wkv = np.asarray(Wkv, dtype=np.float32)
    W1 = np.asarray(W1, dtype=np.float32)
    b1 = np.asarray(b1, dtype=np.float32)
    W2 = np.asarray(W2, dtype=np.float32)
    b2 = np.asarray(b2, dtype=np.float32)

    tri = np.triu(np.ones((128, 128), dtype=np.float32)).astype(BF)

    in_maps = []
    for core in range(N_CORES):
        b, g = core // 2, core % 2
        hs_sl = slice(g * 8, g * 8 + 8)
        w1h = W1[g * 512:(g + 1) * 512]  # [512 hd, 512 j]
        in_maps.append({
            "xq": query_x[b].astype(BF),
            "xkv": np.ascontiguousarray(kv_x[:, b]).astype(BF),
            "wq": _pack_wqk(Wq[hs_sl], scale=HS ** -0.5),
            "wk": np.stack([_pack_wqk(Wkv[m, hs_sl, :, :HS]) for m in range(M3)]),
            "wv": np.stack([_pack_wv(Wkv[m, hs_sl, :, HS:]) for m in range(M3)]),
            "w1": np.ascontiguousarray(
                w1h.reshape(8, 64, 4, 128).transpose(1, 0, 2, 3)
            ).astype(BF),
            "tri": tri,
        })

    res_a = run_bass_kernel_spmd(nc_a, in_maps, core_ids=list(range(N_CORES)))

    # host glue: add the two head-group partials per batch
    w2p = np.ascontiguousarray(W2.reshape(4, 128, 1024).transpose(1, 0, 2)).astype(BF)
    b1s = np.ascontiguousarray(b1.reshape(4, 128).T)
    b2p = b2.reshape(1, 1024).astype(BF)
    in_maps_b = []
    for core in range(N_CORES):
        b, g = core // 2, core % 2
        y1 = (res_a.results[2 * b]["y1p"] + res_a.results[2 * b + 1]["y1p"])  # [4,128,S]
        y1h = np.ascontiguousarray(y1[:, :, g * 1024:(g + 1) * 1024].transpose(1, 0, 2))
        in_maps_b.append({"y1h": y1h, "b1s": b1s, "w2": w2p, "b2": b2p})

    res_b = run_bass_kernel_spmd(nc_b, in_maps_b, core_ids=list(range(N_CORES)))

    out = np.empty((B, S, C), dtype=np.float32)
    for core in range(N_CORES):
        b, g = core // 2, core % 2
        out[b, g * 1024:(g + 1) * 1024, :] = res_b.results[core]["ob"]
    return out

